# revision 1
# baseline (speedup 1.0000x reference)
"""Causal dot-product attention (B=4, S=4096, D=64) on 8 TRN2 NeuronCores.

Returns BOTH outputs of the reference: (attn_vec [B,S,D], attn_weights [B,S,S]).

Sharding: data-parallel over batch (4) x causal-balanced q-row interleave (2).
Core c handles batch b = c//2 and 16 of the 32 128-row q-blocks, chosen as
pairs (i, 31-i) so every core owns the same total causal area (load balance).

SPMD trick: the per-core *program* must be identical, but the causal widths of
a core's blocks differ between the two parities. The program computes padded
uniform widths W_k = 128*(2k+2) for the k-th (sorted) local block and applies a
per-core *data* mask (host-provided) to the last 256 columns, which zeroes the
above-diagonal part exactly. Columns beyond W_k are never written: the PJRT
execution path donates zero-initialized output buffers, so the skipped region
is exactly 0.0 (= reference: exp((x-1e31)/8 - m) underflows to 0).

Per-core pipeline (all engines balanced under the DMA/ACT roofline):
  S^T pass: scores^T tiles [c128, q<=512] = K^T-slice.T @ Q^T (PE, f32)
            -> exp(x/8) -> wT bf16 in SBUF (ACT), boundary masked (DVE)
  PV pass:  vecT[64, 512] += V-slice.T @ wT-slice (PE, bf16), per 512-q window
            -> PE-transpose back to [q,64], scale by 1/rowsum (DVE)
  S pass:   scores tiles [q128, c<=512] = Q^T-slice.T @ K^T (PE, f32)
            -> exp(x/8) f32 with accum_out row-sums (ACT) -> mask (DVE)
            -> scale by 1/rowsum (DVE) -> DMA out active columns only
"""

import numpy as np
from contextlib import ExitStack

import ml_dtypes

from concourse import bacc, tile, mybir, masks
from concourse.bass_utils import run_bass_kernel_spmd

B, S, D = 4, 4096, 64
NCORES = 8
NLOCAL = 16          # 128-row q-blocks per core
SCALE = 0.125        # 1/sqrt(64)

BF16 = mybir.dt.bfloat16
F32 = mybir.dt.float32


def _block_list(parity: int) -> list[int]:
    """Global 128-row block ids owned by a core of this parity, sorted."""
    ids = []
    for i in range(16):
        if i % 2 == parity:
            ids.append(i)
            ids.append(31 - i)
    return sorted(ids)


def _padded_width(k: int) -> int:
    # uniform (parity-independent) padded causal width of local block k
    return 128 * (2 * k + 2)


def _chunks(total: int, first: int | None = None) -> list[tuple[int, int]]:
    """Split [0, total) into (offset, width) chunks of <=512, optionally with a
    fixed-width first chunk."""
    out = []
    o = 0
    if first is not None:
        w = min(first, total)
        out.append((0, w))
        o = w
    while o < total:
        w = min(512, total - o)
        out.append((o, w))
        o += w
    return out


def build_model():
    nc = bacc.Bacc("TRN2", target_bir_lowering=False, debug=False,
                   num_devices=NCORES)

    qt_d = nc.dram_tensor("qt", [D, 2048], F32, kind="ExternalInput").ap()
    kt_d = nc.dram_tensor("kt", [D, S], F32, kind="ExternalInput").ap()
    v_d = nc.dram_tensor("v", [S, D], BF16, kind="ExternalInput").ap()
    smask_d = nc.dram_tensor("smask", [NLOCAL, 128, 256], F32,
                             kind="ExternalInput").ap()
    tmask_d = nc.dram_tensor("tmask", [32, 128, 256], BF16,
                             kind="ExternalInput").ap()
    w_out = nc.dram_tensor("w", [2048, S], F32, kind="ExternalOutput").ap()
    vec_out = nc.dram_tensor("vec", [2048, D], F32, kind="ExternalOutput").ap()

    with tile.TileContext(nc) as tc:
        with ExitStack() as ctx:
            const = ctx.enter_context(tc.tile_pool(name="const", bufs=1))
            wtp = ctx.enter_context(tc.tile_pool(name="wt", bufs=1))
            stg = ctx.enter_context(tc.tile_pool(name="stg", bufs=2))
            vtp = ctx.enter_context(tc.tile_pool(name="vt", bufs=2))
            vcp = ctx.enter_context(tc.tile_pool(name="vc", bufs=2))
            smallp = ctx.enter_context(tc.tile_pool(name="small", bufs=4))
            ps_st = ctx.enter_context(
                tc.tile_pool(name="ps_st", bufs=2, space="PSUM"))
            ps_s = ctx.enter_context(
                tc.tile_pool(name="ps_s", bufs=2, space="PSUM"))
            ps_vt = ctx.enter_context(
                tc.tile_pool(name="ps_vt", bufs=2, space="PSUM"))
            ps_tr = ctx.enter_context(
                tc.tile_pool(name="ps_tr", bufs=2, space="PSUM"))

            # ---- load inputs ----
            qt = const.tile([D, 2048], F32)
            nc.sync.dma_start(qt[:], qt_d[:])
            kt = const.tile([D, S], F32)
            nc.sync.dma_start(kt[:], kt_d[:])
            vs = const.tile([128, 32 * D], BF16)
            nc.sync.dma_start(
                vs[:].rearrange("p (t d) -> p t d", t=32),
                v_d.rearrange("(t p) d -> p t d", p=128),
            )
            sm = const.tile([128, NLOCAL * 256], F32)
            nc.sync.dma_start(
                sm[:].rearrange("p (k j) -> p k j", k=NLOCAL),
                smask_d.rearrange("k p j -> p k j"),
            )
            tm = const.tile([128, 32 * 256], BF16)
            nc.sync.dma_start(
                tm[:].rearrange("p (c j) -> p c j", c=32),
                tmask_d.rearrange("c p j -> p c j"),
            )
            ident = const.tile([128, 128], F32)
            masks.make_identity(nc, ident[:])

            r_all = const.tile([128, NLOCAL], F32)   # 1/rowsum per local block

            wt = {}  # ci -> bf16 tile [128, suffix]

            def emit_st_row(ci: int):
                """S^T pass for c-row ci: wT[ci] = exp(scores^T / 8) (bf16)."""
                kmin = ci // 2
                sfx = 2048 - 128 * kmin
                t = wtp.tile([128, sfx], BF16, tag=f"wt{ci}")
                wt[ci] = t
                bw = min(256, sfx)
                for (o, cw) in _chunks(sfx, first=bw):
                    ps = ps_st.tile([128, 512], F32)
                    nc.tensor.matmul(
                        ps[:, 0:cw],
                        kt[:, 128 * ci:128 * ci + 128],
                        qt[:, 128 * kmin + o:128 * kmin + o + cw],
                        start=True, stop=True,
                    )
                    nc.scalar.activation(
                        t[:, o:o + cw], ps[:, 0:cw],
                        mybir.ActivationFunctionType.Exp, scale=SCALE,
                    )
                # zero/triangle fixup for the two boundary sub-blocks
                nc.vector.tensor_mul(
                    t[:, 0:bw], t[:, 0:bw], tm[:, 256 * ci:256 * ci + bw])

            def emit_s_block(k: int):
                """S pass for local q-block k: normalized weights -> HBM."""
                W = _padded_width(k)
                stage = stg.tile([128, S], F32, tag="stage")
                partials = smallp.tile([128, 12], F32, tag="partials")
                chs = _chunks(W)
                for cc, (o, cw) in enumerate(chs):
                    ps = ps_s.tile([128, 512], F32)
                    nc.tensor.matmul(
                        ps[:, 0:cw],
                        qt[:, 128 * k:128 * k + 128],
                        kt[:, o:o + cw],
                        start=True, stop=True,
                    )
                    nc.scalar.activation(
                        stage[:, o:o + cw], ps[:, 0:cw],
                        mybir.ActivationFunctionType.Exp, scale=SCALE,
                        accum_out=partials[:, cc:cc + 1],
                    )
                # mask last 256 columns (triangle + padding) to exact 0
                nc.vector.tensor_mul(
                    stage[:, W - 256:W], stage[:, W - 256:W],
                    sm[:, 256 * k:256 * k + 256])
                # last chunk's partial included masked garbage; recompute it
                o_l, cw_l = chs[-1]
                nc.vector.reduce_sum(
                    out=partials[:, len(chs) - 1:len(chs)],
                    in_=stage[:, o_l:o_l + cw_l],
                    axis=mybir.AxisListType.X,
                )
                nc.vector.reduce_sum(
                    out=partials[:, 11:12], in_=partials[:, 0:len(chs)],
                    axis=mybir.AxisListType.X,
                )
                nc.vector.reciprocal(r_all[:, k:k + 1], partials[:, 11:12])
                nc.vector.tensor_scalar_mul(
                    stage[:, 0:W], stage[:, 0:W], r_all[:, k:k + 1])
                nc.sync.dma_start(
                    w_out[128 * k:128 * k + 128, 0:W], stage[:, 0:W])

            def emit_pv_window(w: int):
                """PV for local q window [512w, 512w+512): vec rows + DMA."""
                ci_hi = min(31, 8 * w + 7)
                pv = ps_vt.tile([64, 512], F32)
                n_ci = ci_hi + 1
                for ci in range(n_ci):
                    o = 512 * w - 128 * (ci // 2)
                    lhs = vs[:, 64 * ci:64 * ci + 64]
                    if o >= 0:
                        nc.tensor.matmul(
                            pv[:, 0:512], lhs, wt[ci][:, o:o + 512],
                            start=(ci == 0), stop=(ci == n_ci - 1),
                        )
                    else:
                        nc.tensor.matmul(
                            pv[:, -o:512], lhs, wt[ci][:, 0:512 + o],
                            start=False, stop=(ci == n_ci - 1),
                        )
                vt_sb = vtp.tile([64, 512], F32, tag="vtsb")
                nc.vector.tensor_copy(vt_sb[:], pv[:])
                for j in range(4):
                    k = 4 * w + j
                    tr = ps_tr.tile([128, 64], F32)
                    nc.tensor.transpose(
                        tr[:], vt_sb[:, 128 * j:128 * j + 128],
                        ident[0:64, 0:64])
                    vec_sb = vcp.tile([128, 64], F32, tag="vecsb")
                    nc.vector.tensor_scalar_mul(
                        vec_sb[:], tr[:], r_all[:, k:k + 1])
                    nc.sync.dma_start(
                        vec_out[128 * k:128 * k + 128, :], vec_sb[:])

            # ---- emission order: interleave for pipelining ----
            for u in range(NLOCAL):
                emit_st_row(2 * u)
                emit_st_row(2 * u + 1)
                emit_s_block(u)
                if u % 4 == 3:
                    emit_pv_window(u // 4)

    nc.compile()
    return nc


_NC = None


def _get_model():
    global _NC
    if _NC is None:
        _NC = build_model()
    return _NC


def _host_masks(parity: int):
    gl = _block_list(parity)
    smask = np.zeros((NLOCAL, 128, 256), dtype=np.float32)
    for k, g in enumerate(gl):
        W = _padded_width(k)
        cols = W - 256 + np.arange(256)[None, :]
        rows = 128 * g + np.arange(128)[:, None]
        smask[k] = (cols <= rows).astype(np.float32)
    tmask = np.zeros((32, 128, 256), dtype=np.float32)
    for ci in range(32):
        kmin = ci // 2
        bw = min(256, 2048 - 128 * kmin)
        for jj in range(bw // 128):
            k = kmin + jj
            g = gl[k]
            rows_c = 128 * ci + np.arange(128)[:, None]
            cols_q = 128 * g + np.arange(128)[None, :]
            tmask[ci][:, 128 * jj:128 * jj + 128] = (rows_c <= cols_q)
    return smask, tmask.astype(ml_dtypes.bfloat16)


def kernel(query, key, value):
    nc = _get_model()
    query = np.asarray(query, dtype=np.float32)
    key = np.asarray(key, dtype=np.float32)
    value = np.asarray(value, dtype=np.float32)

    mask_cache = {p: _host_masks(p) for p in (0, 1)}
    in_maps = []
    for c in range(NCORES):
        b, p = c // 2, c % 2
        gl = _block_list(p)
        rows = np.concatenate(
            [np.arange(128 * g, 128 * g + 128) for g in gl])
        qt = np.ascontiguousarray(query[b][rows].T)           # [64, 2048]
        kt = np.ascontiguousarray(key[b].T)                   # [64, 4096]
        v = value[b].astype(ml_dtypes.bfloat16)               # [4096, 64]
        smask, tmask = mask_cache[p]
        in_maps.append(
            {"qt": qt, "kt": kt, "v": v, "smask": smask, "tmask": tmask})

    res = run_bass_kernel_spmd(nc, in_maps, core_ids=list(range(NCORES)))

    attn_vec = np.empty((B, S, D), dtype=np.float32)
    attn_w = np.empty((B, S, S), dtype=np.float32)
    for c in range(NCORES):
        b, p = c // 2, c % 2
        gl = _block_list(p)
        wsh = res.results[c]["w"]
        vsh = res.results[c]["vec"]
        for k, g in enumerate(gl):
            attn_w[b, 128 * g:128 * g + 128, :] = wsh[128 * k:128 * k + 128]
            attn_vec[b, 128 * g:128 * g + 128, :] = vsh[128 * k:128 * k + 128]
    return attn_vec, attn_w
